# revision 19
# baseline (speedup 1.0000x reference)
"""Trainium2 Bass kernel for a 3-layer MLP forward pass.

Network: 784 -> 128 (relu) -> 64 (relu) -> 10 (linear), biases folded as the
last row of each weight matrix. Batch 65536, pure data parallel over 8 cores
(8192 rows each).

Strategy:
  * Host pre-transposes each X shard to feature-major layout so the kernel
    streams fully contiguous DMAs and never transposes on-chip.
  * Split-precision matmul: X is sent as an fp16 "hi" plane plus an fp8-e5m2
    residual "lo" plane (3 bytes/elem, 25% less HBM traffic than fp32).
    Layer 1 runs two accumulating PE pass groups: hi@W0_fp16 + lo@W0_e5m2.
    16-bit/8-bit passes stream at 1 cycle/column (exact fp32 runs 2-pass
    LOW_HIGH at ~4x the PE time), which takes the kernel to the HBM memory
    roofline. Measured output error ~5.5e-5 relative (~1e-2 absmax on
    outputs of scale ~100); fp32-exact measures 6e-8 but runs ~2.3x slower.
  * Hidden layers use fp16 activations produced directly by the ScalarEngine
    (relu+bias fused via the activation bias port; final linear bias via
    Identity), so the inter-layer critical path is PE -> ACT -> PE only.
  * Two-stage software pipeline (emit L1 of tile t, L2 of t-1, L3 of t-2)
    with explicit scheduler ordering deps so PE never stalls on ACT.
  * Output is produced feature-major [10, 8192] per core; host transposes back.
"""

import numpy as np
import ml_dtypes

import concourse.bass as bass
import concourse.mybir as mybir
from concourse import bacc
from concourse.bass_utils import run_bass_kernel_spmd
from concourse.tile import TileContext, add_dep_helper

N_CORES = 8
BATCH = 65536
B_SHARD = BATCH // N_CORES  # 8192
F_IN = 784
H1, H2, H3 = 128, 64, 10
KC = 112  # contraction chunk (784 = 7 * 112)
NCHUNK = 7
BT = 512  # batch tile = matmul moving free dim (PSUM bank cap)
NT = B_SHARD // BT
XPLANE = NCHUNK * BT  # 3584 columns per plane

# fp16 const pack: W0 hi chunks + W1 + W2 (columns of a [128, CWH] tile).
# The *B/*R planes hold bf16 bit patterns (bitcast at use): full-W bf16 for the
# h-residual pass and bf16(W - fp16(W)) for the weight-correction pass.
W0H_OFF = 0  # [112, 896]
W1H_OFF = 896  # [128, 64] fp16
W1B_OFF = 960  # [128, 64] bf16 bits
W1R_OFF = 1024  # [128, 64] bf16 residual bits
W2H_OFF = 1088  # [64, 10] fp16
W2B_OFF = 1098  # [64, 10] bf16 bits
W2R_OFF = 1108  # [64, 10] bf16 residual bits
CWH = 1118
# e5m2 const pack: W0 for the lo pass
CWL = 896  # [112, 896]

F32 = mybir.dt.float32
F16 = mybir.dt.float16
F8 = mybir.dt.float8e5
BF16 = mybir.dt.bfloat16
RELU = mybir.ActivationFunctionType.Relu
IDENT = mybir.ActivationFunctionType.Identity


def _build_bass() -> bass.Bass:
    nc = bacc.Bacc()

    xh = nc.dram_tensor("xh", [NT // 2, KC, 2 * XPLANE], F16, kind="ExternalInput")
    xl = nc.dram_tensor("xl", [NT // 2, KC, 2 * XPLANE], F8, kind="ExternalInput")
    wh = nc.dram_tensor("wh", [128, CWH], F16, kind="ExternalInput")
    wl = nc.dram_tensor("wl", [KC, CWL], F8, kind="ExternalInput")
    bias = nc.dram_tensor("bias", [128, 3], F32, kind="ExternalInput")
    y = nc.dram_tensor("y", [H3, B_SHARD], F32, kind="ExternalOutput")

    with TileContext(nc) as tc:
        with (
            tc.tile_pool(name="cp", bufs=1) as cp,
            tc.tile_pool(name="xhp", bufs=5) as xhp,
            tc.tile_pool(name="xlp", bufs=5) as xlp,
            tc.tile_pool(name="hp", bufs=3) as hp,
            tc.tile_pool(name="yp", bufs=3) as yp,
            tc.tile_pool(name="scratchp", bufs=1) as scratchp,
            tc.tile_pool(name="ps1p", bufs=2, space="PSUM") as ps1p,
            tc.tile_pool(name="ps2p", bufs=2, space="PSUM") as ps2p,
            tc.tile_pool(name="ps3p", bufs=2, space="PSUM") as ps3p,
        ):
            wht = cp.tile([128, CWH], F16)
            nc.scalar.dma_start(wht, wh[:])
            wlt = cp.tile([KC, CWL], F8)
            nc.scalar.dma_start(wlt, wl[:])
            bt = cp.tile([128, 3], F32)
            nc.scalar.dma_start(bt, bias[:])

            b0t = bt[0:H1, 0:1]
            b1t = bt[0:H2, 1:2]
            b2t = bt[0:H3, 2:3]

            w0h = lambda c: wht[0:KC, W0H_OFF + c * H1 : W0H_OFF + (c + 1) * H1]
            w0l = lambda c: wlt[0:KC, c * H1 : (c + 1) * H1]
            w1h = wht[0:H1, W1H_OFF : W1H_OFF + H2]
            w1b = wht[0:H1, W1B_OFF : W1B_OFF + H2].bitcast(BF16)
            w1r = wht[0:H1, W1R_OFF : W1R_OFF + H2].bitcast(BF16)
            w2h = wht[0:H2, W2H_OFF : W2H_OFF + H3]
            w2b = wht[0:H2, W2B_OFF : W2B_OFF + H3].bitcast(BF16)
            w2r = wht[0:H2, W2R_OFF : W2R_OFF + H3].bitcast(BF16)

            # absorb const-DMA waits on PE and ACT early
            ps0 = ps1p.tile([H1, BT], F32, tag="ps1")
            nc.tensor.matmul(ps0, lhsT=w0h(0), rhs=wht[0:KC, 0:BT], start=True, stop=True)
            nc.tensor.matmul(ps0, lhsT=w0l(0), rhs=wlt[0:KC, 0:BT], start=True, stop=True)
            actwarm = scratchp.tile([128, 1], F32)
            nc.scalar.activation(actwarm, bt[:, 0:1], RELU)

            # Two-stage software pipeline: iter t emits L1(t), L2(t-1), L3(t-2)
            # with explicit PE-ordering deps so every matmul's ACT input is
            # ready well before PE reaches it.
            h1_of = {}
            h2_of = {}
            last_l1_mm = None
            for t in range(NT + 2):
                prev_l1_mm = last_l1_mm
                if t < NT:
                    if t % 2 == 0:
                        xht = xhp.tile([KC, 2 * XPLANE], F16, tag="xht")
                        xlt = xlp.tile([KC, 2 * XPLANE], F8, tag="xlt")
                        if t == 0:
                            # split the first group so tile 0 lands sooner
                            nc.sync.dma_start(xht[:, :XPLANE], xh[0][:, :XPLANE])
                            nc.sync.dma_start(xlt[:, :XPLANE], xl[0][:, :XPLANE])
                            nc.sync.dma_start(xht[:, XPLANE:], xh[0][:, XPLANE:])
                            nc.sync.dma_start(xlt[:, XPLANE:], xl[0][:, XPLANE:])
                        else:
                            nc.sync.dma_start(xht, xh[t // 2])
                            nc.sync.dma_start(xlt, xl[t // 2])
                        cur_x = (xht, xlt)
                    xht, xlt = cur_x
                    so = (t % 2) * XPLANE

                    ps1 = ps1p.tile([H1, BT], F32, tag="ps1")
                    for c in range(NCHUNK):
                        mm = nc.tensor.matmul(
                            ps1,
                            lhsT=w0h(c),
                            rhs=xht[:, so + c * BT : so + (c + 1) * BT],
                            start=(c == 0),
                            stop=False,
                        )
                    for c in range(NCHUNK):
                        mm = nc.tensor.matmul(
                            ps1,
                            lhsT=w0l(c),
                            rhs=xlt[:, so + c * BT : so + (c + 1) * BT],
                            start=False,
                            stop=(c == NCHUNK - 1),
                        )
                    last_l1_mm = mm
                    h1f = hp.tile([H1, BT], F32, tag="h1f")
                    nc.scalar.activation(h1f, ps1, RELU, bias=b0t)
                    h1h = hp.tile([H1, BT], F16, tag="h1h")
                    nc.vector.tensor_copy(h1h, h1f)
                    h1l = hp.tile([H1, BT], BF16, tag="h1l")
                    nc.vector.tensor_sub(h1l, h1f, h1h)
                    h1b = hp.tile([H1, BT], BF16, tag="h1b")
                    nc.vector.tensor_copy(h1b, h1f)
                    h1_of[t] = (h1h, h1l, h1b)

                if t >= 1 and t - 1 < NT:
                    h1h, h1l, h1b = h1_of.pop(t - 1)
                    ps2 = ps2p.tile([H2, BT], F32, tag="ps2")
                    mm2 = nc.tensor.matmul(ps2, lhsT=w1h, rhs=h1h, start=True, stop=False)
                    if prev_l1_mm is not None:
                        add_dep_helper(mm2.ins, prev_l1_mm.ins, sync=False,
                                       reason="pipeline: L2(t-1) after L1(t)")
                    nc.tensor.matmul(ps2, lhsT=w1b, rhs=h1l, start=False, stop=False)
                    nc.tensor.matmul(ps2, lhsT=w1r, rhs=h1b, start=False, stop=True)
                    h2f = hp.tile([H2, BT], F32, tag="h2f")
                    nc.scalar.activation(h2f, ps2, RELU, bias=b1t)
                    h2h = hp.tile([H2, BT], F16, tag="h2h")
                    nc.vector.tensor_copy(h2h, h2f)
                    h2l = hp.tile([H2, BT], BF16, tag="h2l")
                    nc.vector.tensor_sub(h2l, h2f, h2h)
                    h2b = hp.tile([H2, BT], BF16, tag="h2b")
                    nc.vector.tensor_copy(h2b, h2f)
                    h2_of[t - 1] = (h2h, h2l, h2b)

                if t >= 2:
                    h2h, h2l, h2b = h2_of.pop(t - 2)
                    ps3 = ps3p.tile([H3, BT], F32, tag="ps3")
                    mm3 = nc.tensor.matmul(ps3, lhsT=w2h, rhs=h2h, start=True, stop=False)
                    if prev_l1_mm is not None:
                        add_dep_helper(mm3.ins, prev_l1_mm.ins, sync=False,
                                       reason="pipeline: L3(t-2) after L1(t)")
                    nc.tensor.matmul(ps3, lhsT=w2b, rhs=h2l, start=False, stop=False)
                    nc.tensor.matmul(ps3, lhsT=w2r, rhs=h2b, start=False, stop=True)
                    yt = yp.tile([H3, BT], F32, tag="yt")
                    nc.scalar.activation(yt, ps3, IDENT, bias=b2t)
                    nc.gpsimd.dma_start(y[:, (t - 2) * BT : (t - 1) * BT], yt)

    nc.finalize()
    return nc


_CACHED_NC: bass.Bass | None = None


def _get_nc() -> bass.Bass:
    global _CACHED_NC
    if _CACHED_NC is None:
        _CACHED_NC = _build_bass()
    return _CACHED_NC


def _feat_major(a: np.ndarray) -> np.ndarray:
    """[8192, 784] batch-major -> [NT/2, KC, 2*NCHUNK*BT] feature-major pairs."""
    fm = (
        np.ascontiguousarray(a.reshape(NT, BT, NCHUNK, KC).transpose(0, 3, 2, 1))
        .reshape(NT, KC, XPLANE)
    )
    return (
        np.ascontiguousarray(fm.reshape(NT // 2, 2, KC, XPLANE).transpose(0, 2, 1, 3))
        .reshape(NT // 2, KC, 2 * XPLANE)
    )


def _chunked_w0(a):
    return a.reshape(NCHUNK, KC, H1).transpose(1, 0, 2).reshape(KC, NCHUNK * H1)


def kernel(X: np.ndarray, W0: np.ndarray, W1: np.ndarray, W2: np.ndarray, **_kw):
    X = np.ascontiguousarray(X, dtype=np.float32)
    W0 = np.asarray(W0, dtype=np.float32)
    W1 = np.asarray(W1, dtype=np.float32)
    W2 = np.asarray(W2, dtype=np.float32)

    wh16 = np.zeros((128, CWH), dtype=np.uint16)
    wh16[0:KC, W0H_OFF : W0H_OFF + 896] = _chunked_w0(
        W0[:F_IN].astype(np.float16)
    ).view(np.uint16)
    bfv = lambda a: a.astype(ml_dtypes.bfloat16).view(np.uint16)
    f16v = lambda a: a.astype(np.float16).view(np.uint16)
    w1m, w2m = W1[:H1], W2[:H2]
    wh16[0:H1, W1H_OFF : W1H_OFF + H2] = f16v(w1m)
    wh16[0:H1, W1B_OFF : W1B_OFF + H2] = bfv(w1m)
    wh16[0:H1, W1R_OFF : W1R_OFF + H2] = bfv(w1m - w1m.astype(np.float16).astype(np.float32))
    wh16[0:H2, W2H_OFF : W2H_OFF + H3] = f16v(w2m)
    wh16[0:H2, W2B_OFF : W2B_OFF + H3] = bfv(w2m)
    wh16[0:H2, W2R_OFF : W2R_OFF + H3] = bfv(w2m - w2m.astype(np.float16).astype(np.float32))
    wh = wh16.view(np.float16)
    wlo = _chunked_w0(W0[:F_IN].astype(ml_dtypes.float8_e5m2))

    biases = np.zeros((128, 3), dtype=np.float32)
    biases[0:H1, 0] = W0[F_IN]
    biases[0:H2, 1] = W1[H1]
    biases[0:H3, 2] = W2[H2]

    Xh = X.astype(np.float16)
    Xl = (X - Xh.astype(np.float32)).astype(ml_dtypes.float8_e5m2)

    in_maps = []
    for c in range(N_CORES):
        sl = slice(c * B_SHARD, (c + 1) * B_SHARD)
        in_maps.append(
            {
                "xh": _feat_major(Xh[sl]),
                "xl": _feat_major(Xl[sl]),
                "wh": wh,
                "wl": wlo,
                "bias": biases,
            }
        )

    res = run_bass_kernel_spmd(_get_nc(), in_maps, core_ids=list(range(N_CORES)))
    global LAST_RESULT
    LAST_RESULT = res
    out = np.concatenate([r["y"].T for r in res.results], axis=0)
    return np.ascontiguousarray(out)


LAST_RESULT = None


if __name__ == "__main__":
    rng = np.random.default_rng(0)
    X = rng.standard_normal((BATCH, F_IN), dtype=np.float32)
    W0 = rng.random((F_IN + 1, H1), dtype=np.float32) * 0.1
    W1 = rng.random((H1 + 1, H2), dtype=np.float32) * 0.1
    W2 = rng.random((H2 + 1, H3), dtype=np.float32) * 0.1
    y = kernel(X=X, W0=W0, W1=W1, W2=W2)
    print(y.shape, y.dtype, y[:2])


# revision 20
# speedup vs baseline: 1.0938x; 1.0938x over previous
"""Trainium2 Bass kernel for a 3-layer MLP forward pass.

Network: 784 -> 128 (relu) -> 64 (relu) -> 10 (linear), biases folded as the
last row of each weight matrix. Batch 65536, pure data parallel over 8 cores
(8192 rows each).

Strategy:
  * Host pre-transposes each X shard to feature-major layout so the kernel
    streams fully contiguous DMAs and never transposes on-chip.
  * Split-precision matmul: X is sent as an fp16 "hi" plane plus an fp8-e5m2
    residual "lo" plane (3 bytes/elem, 25% less HBM traffic than fp32).
    Layer 1 runs two accumulating PE pass groups: hi@W0_fp16 + lo@W0_e5m2.
    16-bit/8-bit passes stream at 1 cycle/column (exact fp32 runs 2-pass
    LOW_HIGH at ~4x the PE time), which takes the kernel to the HBM memory
    roofline. Measured output error ~5.5e-5 relative (~1e-2 absmax on
    outputs of scale ~100); fp32-exact measures 6e-8 but runs ~2.3x slower.
  * Hidden layers use fp16 activations produced directly by the ScalarEngine
    (relu+bias fused via the activation bias port; final linear bias via
    Identity), so the inter-layer critical path is PE -> ACT -> PE only.
  * Two-stage software pipeline (emit L1 of tile t, L2 of t-1, L3 of t-2)
    with explicit scheduler ordering deps so PE never stalls on ACT.
  * Output is produced feature-major [10, 8192] per core; host transposes back.
"""

import numpy as np
import ml_dtypes

import concourse.bass as bass
import concourse.mybir as mybir
from concourse import bacc
from concourse.bass_utils import run_bass_kernel_spmd
from concourse.tile import TileContext, add_dep_helper

N_CORES = 8
BATCH = 65536
B_SHARD = BATCH // N_CORES  # 8192
F_IN = 784
H1, H2, H3 = 128, 64, 10
KC = 112  # contraction chunk (784 = 7 * 112)
NCHUNK = 7
BT = 512  # batch tile = matmul moving free dim (PSUM bank cap)
NT = B_SHARD // BT
XPLANE = NCHUNK * BT  # 3584 columns per plane

# fp16 const pack: W0 hi chunks + W1 + W2 (columns of a [128, CWH] tile).
# The *B/*R planes hold bf16 bit patterns (bitcast at use): full-W bf16 for the
# h-residual pass and bf16(W - fp16(W)) for the weight-correction pass.
W0H_OFF = 0  # [112, 896]
W1H_OFF = 896  # [128, 64] fp16
W1B_OFF = 960  # [128, 64] bf16 bits
W1R_OFF = 1024  # [128, 64] bf16 residual bits
W2H_OFF = 1088  # [64, 10] fp16
W2B_OFF = 1098  # [64, 10] bf16 bits
W2R_OFF = 1108  # [64, 10] bf16 residual bits
CWH = 1118
# e5m2 const pack: W0 for the lo pass
CWL = 896  # [112, 896]

F32 = mybir.dt.float32
F16 = mybir.dt.float16
F8 = mybir.dt.float8e5
BF16 = mybir.dt.bfloat16
RELU = mybir.ActivationFunctionType.Relu
IDENT = mybir.ActivationFunctionType.Identity


def _build_bass() -> bass.Bass:
    nc = bacc.Bacc()

    xh = nc.dram_tensor("xh", [NT // 2, KC, 2 * XPLANE], F16, kind="ExternalInput")
    xl = nc.dram_tensor("xl", [NT // 2, KC, 2 * XPLANE], F8, kind="ExternalInput")
    wh = nc.dram_tensor("wh", [128, CWH], F16, kind="ExternalInput")
    wl = nc.dram_tensor("wl", [KC, CWL], F8, kind="ExternalInput")
    bias = nc.dram_tensor("bias", [128, 3], F32, kind="ExternalInput")
    y = nc.dram_tensor("y", [H3, B_SHARD], F32, kind="ExternalOutput")

    with TileContext(nc) as tc:
        with (
            tc.tile_pool(name="cp", bufs=1) as cp,
            tc.tile_pool(name="xhp", bufs=5) as xhp,
            tc.tile_pool(name="xlp", bufs=5) as xlp,
            tc.tile_pool(name="hp", bufs=3) as hp,
            tc.tile_pool(name="yp", bufs=3) as yp,
            tc.tile_pool(name="scratchp", bufs=1) as scratchp,
            tc.tile_pool(name="ps1p", bufs=2, space="PSUM") as ps1p,
            tc.tile_pool(name="ps2p", bufs=2, space="PSUM") as ps2p,
            tc.tile_pool(name="ps3p", bufs=2, space="PSUM") as ps3p,
        ):
            wht = cp.tile([128, CWH], F16)
            nc.scalar.dma_start(wht, wh[:])
            wlt = cp.tile([KC, CWL], F8)
            nc.scalar.dma_start(wlt, wl[:])
            bt = cp.tile([128, 3], F32)
            nc.scalar.dma_start(bt, bias[:])

            b0t = bt[0:H1, 0:1]
            b1t = bt[0:H2, 1:2]
            b2t = bt[0:H3, 2:3]

            w0h = lambda c: wht[0:KC, W0H_OFF + c * H1 : W0H_OFF + (c + 1) * H1]
            w0l = lambda c: wlt[0:KC, c * H1 : (c + 1) * H1]
            w1h = wht[0:H1, W1H_OFF : W1H_OFF + H2]
            w1b = wht[0:H1, W1B_OFF : W1B_OFF + H2].bitcast(BF16)
            w1r = wht[0:H1, W1R_OFF : W1R_OFF + H2].bitcast(BF16)
            w2h = wht[0:H2, W2H_OFF : W2H_OFF + H3]
            w2b = wht[0:H2, W2B_OFF : W2B_OFF + H3].bitcast(BF16)
            w2r = wht[0:H2, W2R_OFF : W2R_OFF + H3].bitcast(BF16)

            # absorb const-DMA waits on PE and ACT early
            ps0 = ps1p.tile([H1, BT], F32, tag="ps1")
            nc.tensor.matmul(ps0, lhsT=w0h(0), rhs=wht[0:KC, 0:BT], start=True, stop=True)
            nc.tensor.matmul(ps0, lhsT=w0l(0), rhs=wlt[0:KC, 0:BT], start=True, stop=True)
            actwarm = scratchp.tile([128, 1], F32)
            nc.scalar.activation(actwarm, bt[:, 0:1], RELU)

            # Two-stage software pipeline: iter t emits L1(t), L2(t-1), L3(t-2)
            # with explicit PE-ordering deps so every matmul's ACT input is
            # ready well before PE reaches it.
            h1_of = {}
            h2_of = {}
            last_l1_mm = None
            for t in range(NT + 2):
                prev_l1_mm = last_l1_mm
                if t < NT:
                    if t % 2 == 0:
                        xht = xhp.tile([KC, 2 * XPLANE], F16, tag="xht")
                        nc.sync.dma_start(xht, xh[t // 2])
                        xlt = xlp.tile([KC, 2 * XPLANE], F8, tag="xlt")
                        nc.sync.dma_start(xlt, xl[t // 2])
                        cur_x = (xht, xlt)
                    xht, xlt = cur_x
                    so = (t % 2) * XPLANE

                    ps1 = ps1p.tile([H1, BT], F32, tag="ps1")
                    for c in range(NCHUNK):
                        mm = nc.tensor.matmul(
                            ps1,
                            lhsT=w0h(c),
                            rhs=xht[:, so + c * BT : so + (c + 1) * BT],
                            start=(c == 0),
                            stop=False,
                        )
                    for c in range(NCHUNK):
                        mm = nc.tensor.matmul(
                            ps1,
                            lhsT=w0l(c),
                            rhs=xlt[:, so + c * BT : so + (c + 1) * BT],
                            start=False,
                            stop=(c == NCHUNK - 1),
                        )
                    last_l1_mm = mm
                    h1f = hp.tile([H1, BT], F32, tag="h1f")
                    nc.scalar.activation(h1f, ps1, RELU, bias=b0t)
                    h1h = hp.tile([H1, BT], F16, tag="h1h")
                    nc.vector.tensor_copy(h1h, h1f)
                    h1l = hp.tile([H1, BT], BF16, tag="h1l")
                    nc.vector.tensor_sub(h1l, h1f, h1h)
                    h1b = hp.tile([H1, BT], BF16, tag="h1b")
                    nc.vector.tensor_copy(h1b, h1f)
                    h1_of[t] = (h1h, h1l, h1b)

                if t >= 1 and t - 1 < NT:
                    h1h, h1l, h1b = h1_of.pop(t - 1)
                    ps2 = ps2p.tile([H2, BT], F32, tag="ps2")
                    mm2 = nc.tensor.matmul(ps2, lhsT=w1h, rhs=h1h, start=True, stop=False)
                    if prev_l1_mm is not None:
                        add_dep_helper(mm2.ins, prev_l1_mm.ins, sync=False,
                                       reason="pipeline: L2(t-1) after L1(t)")
                    nc.tensor.matmul(ps2, lhsT=w1b, rhs=h1l, start=False, stop=False)
                    nc.tensor.matmul(ps2, lhsT=w1r, rhs=h1b, start=False, stop=True)
                    h2f = hp.tile([H2, BT], F32, tag="h2f")
                    nc.scalar.activation(h2f, ps2, RELU, bias=b1t)
                    h2h = hp.tile([H2, BT], F16, tag="h2h")
                    nc.vector.tensor_copy(h2h, h2f)
                    h2l = hp.tile([H2, BT], BF16, tag="h2l")
                    nc.vector.tensor_sub(h2l, h2f, h2h)
                    h2b = hp.tile([H2, BT], BF16, tag="h2b")
                    nc.vector.tensor_copy(h2b, h2f)
                    h2_of[t - 1] = (h2h, h2l, h2b)

                if t >= 2:
                    h2h, h2l, h2b = h2_of.pop(t - 2)
                    ps3 = ps3p.tile([H3, BT], F32, tag="ps3")
                    mm3 = nc.tensor.matmul(ps3, lhsT=w2h, rhs=h2h, start=True, stop=False)
                    if prev_l1_mm is not None:
                        add_dep_helper(mm3.ins, prev_l1_mm.ins, sync=False,
                                       reason="pipeline: L3(t-2) after L1(t)")
                    nc.tensor.matmul(ps3, lhsT=w2b, rhs=h2l, start=False, stop=False)
                    nc.tensor.matmul(ps3, lhsT=w2r, rhs=h2b, start=False, stop=True)
                    yt = yp.tile([H3, BT], F32, tag="yt")
                    nc.scalar.activation(yt, ps3, IDENT, bias=b2t)
                    nc.gpsimd.dma_start(y[:, (t - 2) * BT : (t - 1) * BT], yt)

    nc.finalize()
    return nc


_CACHED_NC: bass.Bass | None = None


def _get_nc() -> bass.Bass:
    global _CACHED_NC
    if _CACHED_NC is None:
        _CACHED_NC = _build_bass()
    return _CACHED_NC


def _feat_major(a: np.ndarray) -> np.ndarray:
    """[8192, 784] batch-major -> [NT/2, KC, 2*NCHUNK*BT] feature-major pairs."""
    fm = (
        np.ascontiguousarray(a.reshape(NT, BT, NCHUNK, KC).transpose(0, 3, 2, 1))
        .reshape(NT, KC, XPLANE)
    )
    return (
        np.ascontiguousarray(fm.reshape(NT // 2, 2, KC, XPLANE).transpose(0, 2, 1, 3))
        .reshape(NT // 2, KC, 2 * XPLANE)
    )


def _chunked_w0(a):
    return a.reshape(NCHUNK, KC, H1).transpose(1, 0, 2).reshape(KC, NCHUNK * H1)


def kernel(X: np.ndarray, W0: np.ndarray, W1: np.ndarray, W2: np.ndarray, **_kw):
    X = np.ascontiguousarray(X, dtype=np.float32)
    W0 = np.asarray(W0, dtype=np.float32)
    W1 = np.asarray(W1, dtype=np.float32)
    W2 = np.asarray(W2, dtype=np.float32)

    wh16 = np.zeros((128, CWH), dtype=np.uint16)
    wh16[0:KC, W0H_OFF : W0H_OFF + 896] = _chunked_w0(
        W0[:F_IN].astype(np.float16)
    ).view(np.uint16)
    bfv = lambda a: a.astype(ml_dtypes.bfloat16).view(np.uint16)
    f16v = lambda a: a.astype(np.float16).view(np.uint16)
    w1m, w2m = W1[:H1], W2[:H2]
    wh16[0:H1, W1H_OFF : W1H_OFF + H2] = f16v(w1m)
    wh16[0:H1, W1B_OFF : W1B_OFF + H2] = bfv(w1m)
    wh16[0:H1, W1R_OFF : W1R_OFF + H2] = bfv(w1m - w1m.astype(np.float16).astype(np.float32))
    wh16[0:H2, W2H_OFF : W2H_OFF + H3] = f16v(w2m)
    wh16[0:H2, W2B_OFF : W2B_OFF + H3] = bfv(w2m)
    wh16[0:H2, W2R_OFF : W2R_OFF + H3] = bfv(w2m - w2m.astype(np.float16).astype(np.float32))
    wh = wh16.view(np.float16)
    wlo = _chunked_w0(W0[:F_IN].astype(ml_dtypes.float8_e5m2))

    biases = np.zeros((128, 3), dtype=np.float32)
    biases[0:H1, 0] = W0[F_IN]
    biases[0:H2, 1] = W1[H1]
    biases[0:H3, 2] = W2[H2]

    Xh = X.astype(np.float16)
    Xl = (X - Xh.astype(np.float32)).astype(ml_dtypes.float8_e5m2)

    in_maps = []
    for c in range(N_CORES):
        sl = slice(c * B_SHARD, (c + 1) * B_SHARD)
        in_maps.append(
            {
                "xh": _feat_major(Xh[sl]),
                "xl": _feat_major(Xl[sl]),
                "wh": wh,
                "wl": wlo,
                "bias": biases,
            }
        )

    res = run_bass_kernel_spmd(_get_nc(), in_maps, core_ids=list(range(N_CORES)))
    global LAST_RESULT
    LAST_RESULT = res
    out = np.concatenate([r["y"].T for r in res.results], axis=0)
    return np.ascontiguousarray(out)


LAST_RESULT = None


if __name__ == "__main__":
    rng = np.random.default_rng(0)
    X = rng.standard_normal((BATCH, F_IN), dtype=np.float32)
    W0 = rng.random((F_IN + 1, H1), dtype=np.float32) * 0.1
    W1 = rng.random((H1 + 1, H2), dtype=np.float32) * 0.1
    W2 = rng.random((H2 + 1, H3), dtype=np.float32) * 0.1
    y = kernel(X=X, W0=W0, W1=W1, W2=W2)
    print(y.shape, y.dtype, y[:2])
